# revision 6
# baseline (speedup 1.0000x reference)
"""3x3 valid conv (cross-correlation) + bias on a 4096x4096 fp32 image,
run across 8 trn2 NeuronCores.

Strategy
--------
Rows are sharded across the 8 cores host-side with a 2-row halo folded
into each core's input slice (no device collectives needed). On each
core the conv is computed as banded matmuls on the TensorEngine:

  For an output row-tile of M=126 rows (input rows K=M+2), and each of
  the 3 kernel columns dj, build a banded stationary matrix
  B_dj[k, m] = w[k-m, dj] (zero outside 0<=k-m<=2). Then

      Y_tile[m, n] = sum_dj sum_k B_dj[k, m] * X_tile[k, n+dj]

  i.e. 3 matmuls accumulating in PSUM per 512-wide column chunk, with
  the dj shift expressed in the rhs access pattern. Bias is added during
  the PSUM->SBUF copy on the VectorEngine (DMA cannot read PSUM).

The band matrices depend only on the 3x3 weight and are built host-side
and fed as a small extra input.

MM_DTYPE selects the TensorEngine dtype: "f32" is exact but streams at
4 cycles/row; "f32r" streams at 1 cycle/row for N>=256.
"""

import numpy as np

H = 4096
W = 4096
KH = 3
KW = 3
HOUT = H - KH + 1  # 4094
WOUT = W - KW + 1  # 4094
NCORES = 8
ROWS_PER_CORE = 512          # output rows computed per core
IN_ROWS = ROWS_PER_CORE + 2  # input rows per core (with halo)
# Core 7 overlaps core 6 by 2 rows so that all shards have equal shape.
STARTS = [0, 512, 1024, 1536, 2048, 2560, 3072, 3582]
M_TILE = 126                 # output rows per matmul tile (K = M + 2 <= 128)
N_CHUNK = 512                # PSUM bank = 512 fp32

MM_DTYPE = "f32r"

_PROGRAM_CACHE = {}


def _build_program(mm_dtype: str):
    import concourse.mybir as mybir
    from concourse import bacc
    from concourse.tile import TileContext

    nc = bacc.Bacc()
    f32 = mybir.dt.float32
    mm_dt = {"f32": f32, "f32r": mybir.dt.float32r}[mm_dtype]
    x = nc.declare_dram_parameter("x", [IN_ROWS, W], mm_dt, isOutput=False)
    bands = nc.declare_dram_parameter(
        "bands", [128, KW, M_TILE], mm_dt, isOutput=False
    )
    biasc = nc.declare_dram_parameter("biasc", [128, 1], f32, isOutput=False)
    y = nc.declare_dram_parameter("y", [ROWS_PER_CORE, WOUT], f32, isOutput=True)

    n_tiles = (ROWS_PER_CORE + M_TILE - 1) // M_TILE
    chunks = []
    n0 = 0
    while n0 < WOUT:
        chunks.append((n0, min(N_CHUNK, WOUT - n0)))
        n0 += N_CHUNK

    with TileContext(nc) as tc:
        with (
            tc.tile_pool(name="consts", bufs=1) as consts,
            tc.tile_pool(name="xp", bufs=3) as xp,
            tc.tile_pool(name="yp", bufs=2) as yp,
            tc.tile_pool(name="pp", bufs=8, space="PSUM") as pp,
        ):
            bands_sb = consts.tile([128, KW, M_TILE], mm_dt)
            nc.sync.dma_start(out=bands_sb[:], in_=bands[:])
            bias_sb = consts.tile([128, 1], f32)
            nc.sync.dma_start(out=bias_sb[:], in_=biasc[:])

            for t in range(n_tiles):
                r0 = t * M_TILE
                m = min(M_TILE, ROWS_PER_CORE - r0)
                k = m + KH - 1
                xt = xp.tile([128, W], mm_dt, tag="xt")
                nc.sync.dma_start(out=xt[:k, :], in_=x[r0 : r0 + k, :])
                yt = yp.tile([128, WOUT], f32, tag="yt")
                for n0, n in chunks:
                    pt = pp.tile([M_TILE, N_CHUNK], f32, tag="pt")
                    for dj in range(KW):
                        lhsT = bands_sb[:k, dj, :m]
                        rhs = xt[:k, n0 + dj : n0 + dj + n]
                        nc.tensor.matmul(
                            pt[:m, :n],
                            lhsT,
                            rhs,
                            start=(dj == 0),
                            stop=(dj == KW - 1),
                        )
                    nc.vector.tensor_scalar_add(
                        yt[:m, n0 : n0 + n], pt[:m, :n], bias_sb[:m, :]
                    )
                nc.sync.dma_start(out=y[r0 : r0 + m, :], in_=yt[:m, :])
    nc.finalize()
    return nc


def _get_program(mm_dtype: str):
    if mm_dtype not in _PROGRAM_CACHE:
        _PROGRAM_CACHE[mm_dtype] = _build_program(mm_dtype)
    return _PROGRAM_CACHE[mm_dtype]


def _make_bands(w: np.ndarray) -> np.ndarray:
    bands = np.zeros((128, KW, M_TILE), np.float32)
    idx = np.arange(M_TILE)
    for dj in range(KW):
        for d in range(KH):
            bands[idx + d, dj, idx] = w[d, dj]
    return bands


def _run(X, weight, bias, trace=False, mm_dtype=None):
    from concourse.bass_utils import run_bass_kernel_spmd

    mm_dtype = mm_dtype or MM_DTYPE
    X = np.ascontiguousarray(np.asarray(X, dtype=np.float32))
    w = np.asarray(weight, dtype=np.float32)
    b = np.asarray(bias, dtype=np.float32)
    assert X.shape == (H, W) and w.shape == (KH, KW)

    nc = _get_program(mm_dtype)
    bands = _make_bands(w)
    biasc = np.full((128, 1), b[0], np.float32)
    in_maps = [
        {"x": X[s : s + IN_ROWS], "bands": bands, "biasc": biasc} for s in STARTS
    ]
    res = run_bass_kernel_spmd(
        nc, in_maps, core_ids=list(range(NCORES)), trace=trace
    )
    out = np.empty((HOUT, WOUT), np.float32)
    for c in range(NCORES - 1):
        out[STARTS[c] : STARTS[c] + ROWS_PER_CORE] = res.results[c]["y"]
    out[STARTS[-1] + 2 :] = res.results[-1]["y"][2:]
    return out, res.exec_time_ns


def kernel(X, weight, bias):
    out, _ = _run(X, weight, bias, trace=False)
    return out
